# revision 6
# baseline (speedup 1.0000x reference)
"""Trainium2 Bass kernel for nn_Encoder_88235808129468 (scatter_memory).

reference semantics:
    proj = relu(emb @ W + b)                      # [B, N, 32]
    proj *= (n < entity_num[b])                   # mask padded entities
    out[b, :, y, x] += proj[b, n, :]              # scatter-add into [B, 32, H, W]

Strategy (pure data-parallel over batch, 8 cores x 8 batches):
  - ExternalOutput DRAM buffers are pre-zeroed by the PJRT runner
    (documented contract in bass2jax.run_bass_via_pjrt), so the kernel only
    writes the scattered entity rows; untouched cells stay zero.
  - Device output layout is HW-major [HW, 32] rows at 256B stride (the
    dma_scatter_add ucode requires a 256B-multiple row stride); the host
    reorders axes to [B, 32, H, W] at the end.
  - Duplicate scatter indices race across the 16 SDMA engines (CCE
    read-modify-writes are not serialized between engines), so duplicates are
    eliminated on device: a 512x512 selection matrix (idx_i == idx_j) feeds a
    matmul that gives the FIRST entity of each duplicate group the full group
    total; all later duplicates are redirected to a trash row.
  - Masked (padded) entities are zeroed before the group-total matmul, so
    they contribute nothing, and whichever group member survives dedup writes
    the correct total.
"""
import os
import sys
import types

sys.path.insert(0, "/opt/trn_rl_repo")

import numpy as np


def _install_axon_hooks_stub():
    """bass_utils imports antenv.axon_hooks when tracing; give it a no-op."""
    if "antenv.axon_hooks" in sys.modules:
        return
    mod = types.ModuleType("antenv.axon_hooks")
    _state = {"hook": None}
    mod.set_axon_ntff_profile_hook = lambda h: _state.__setitem__("hook", h)
    mod.get_axon_ntff_profile_hook = lambda: _state["hook"]
    sys.modules["antenv.axon_hooks"] = mod


_install_axon_hooks_stub()

from concourse import bass, mybir, bacc  # noqa: E402
from concourse.bass_utils import run_bass_kernel_spmd  # noqa: E402
from concourse.masks import make_identity  # noqa: E402
import concourse.tile as tile  # noqa: E402

# Problem constants (hardcoded per harness contract)
B, N, D_IN, D_OUT, H, W = 64, 512, 256, 160, 160, 160  # noqa: E741  (D_OUT=32 below)
D_OUT = 32
HH, WW = 160, 160
HW = HH * WW           # 25600
NCORES = 8
BPC = B // NCORES      # 8 batches per core
NCH = N // 128         # 4 entity chunks of 128
TRASH = HW             # duplicate entities scatter here
VROWS = HW + 8         # 25608 rows (trash row + pad)
ESTEP = 64             # out row stride in f32 elems (256B, ucode requirement)
F32 = mybir.dt.float32
I32 = mybir.dt.int32
I16 = mybir.dt.int16

_NC_CACHE = None


def build_nc():
    nc = bacc.Bacc("TRN2", target_bir_lowering=False, debug=False, num_devices=NCORES)

    embT = nc.dram_tensor("embT", [BPC, D_IN, N], F32, kind="ExternalInput")
    xc = nc.dram_tensor("xc", [BPC, 128, NCH], I32, kind="ExternalInput")
    yc = nc.dram_tensor("yc", [BPC, 128, NCH], I32, kind="ExternalInput")
    wgt = nc.dram_tensor("wgt", [D_IN, D_OUT], F32, kind="ExternalInput")
    bias = nc.dram_tensor("bias", [1, D_OUT], F32, kind="ExternalInput")
    entn = nc.dram_tensor("entn", [BPC, 1], I32, kind="ExternalInput")
    outs = [
        nc.dram_tensor(f"out{b}", [VROWS, ESTEP], F32, kind="ExternalOutput")
        for b in range(BPC)
    ]

    with tile.TileContext(nc) as tc:
        with (
            tc.tile_pool(name="const", bufs=1) as cpool,
            tc.tile_pool(name="io", bufs=2) as iopool,
            tc.tile_pool(name="work", bufs=2) as wpool,
            tc.tile_pool(name="ppool", bufs=2, space="PSUM") as ppool,
            tc.tile_pool(name="ppool1", bufs=1, space="PSUM") as ppool1,
        ):
            # ---- per-core constants ----
            id128 = cpool.tile([128, 128], F32, tag="id128")
            make_identity(nc, id128[:])
            id32 = cpool.tile([32, 32], F32, tag="id32")
            make_identity(nc, id32[:])
            id8 = cpool.tile([8, 8], F32, tag="id8")
            make_identity(nc, id8[:])

            ones_row = cpool.tile([1, 128], F32, tag="ones_row")
            nc.vector.memset(ones_row[:], 1.0)
            ones_col = cpool.tile([128, 1], F32, tag="ones_col")
            nc.vector.memset(ones_col[:], 1.0)
            trash_row = cpool.tile([1, N], F32, tag="trash_row")
            nc.vector.memset(trash_row[:], float(TRASH))

            w0 = cpool.tile([128, D_OUT], F32, tag="w0")
            w1 = cpool.tile([128, D_OUT], F32, tag="w1")
            nc.sync.dma_start(out=w0[:], in_=wgt[0:128, :])
            nc.sync.dma_start(out=w1[:], in_=wgt[128:256, :])
            bias_t = cpool.tile([1, D_OUT], F32, tag="bias_t")
            nc.sync.dma_start(out=bias_t[:], in_=bias[:, :])

            # entity_num -> per-partition mask tiles, one column per batch
            entn_t = cpool.tile([BPC, 1], I32, tag="entn_t")
            nc.sync.dma_start(out=entn_t[:], in_=entn[:, :])
            entn_f = cpool.tile([BPC, 1], F32, tag="entn_f")
            nc.vector.tensor_copy(out=entn_f[:], in_=entn_t[:])
            entnb_ps = ppool1.tile([128, BPC], F32, tag="misc_ps")
            nc.tensor.transpose(
                out=entnb_ps[:], in_=entn_f[:].to_broadcast([BPC, 128]), identity=id8[:]
            )
            entnb = cpool.tile([128, BPC], F32, tag="entnb")
            nc.vector.tensor_copy(out=entnb[:], in_=entnb_ps[:])

            mask_tiles = []
            for c in range(NCH):
                iota_c = cpool.tile([128, BPC], F32, tag=f"iota{c}")
                nc.gpsimd.iota(
                    iota_c[:], pattern=[[0, BPC]], base=c * 128,
                    channel_multiplier=1, allow_small_or_imprecise_dtypes=True,
                )
                m = cpool.tile([128, BPC], F32, tag=f"mask{c}")
                nc.vector.tensor_tensor(
                    out=m[:], in0=iota_c[:], in1=entnb[:], op=mybir.AluOpType.is_lt
                )
                mask_tiles.append(m)

            # ---- per-batch pipeline ----
            for b in range(BPC):
                e0 = iopool.tile([128, N], F32, tag="embT0")
                e1 = iopool.tile([128, N], F32, tag="embT1")
                nc.sync.dma_start(out=e0[:], in_=embT[b, 0:128, :])
                nc.sync.dma_start(out=e1[:], in_=embT[b, 128:256, :])
                xt = iopool.tile([128, NCH], I32, tag="xt")
                yt = iopool.tile([128, NCH], I32, tag="yt")
                nc.sync.dma_start(out=xt[:], in_=xc[b, :, :])
                nc.sync.dma_start(out=yt[:], in_=yc[b, :, :])

                # flat idx = y*W + x  (chunk layout [128, 4])
                idx_i = wpool.tile([128, NCH], I32, tag="idx_i")
                nc.vector.tensor_scalar(
                    out=idx_i[:], in0=yt[:], scalar1=WW, scalar2=None,
                    op0=mybir.AluOpType.mult,
                )
                nc.vector.tensor_tensor(
                    out=idx_i[:], in0=idx_i[:], in1=xt[:], op=mybir.AluOpType.add
                )
                idx_f = wpool.tile([128, NCH], F32, tag="idx_f")
                nc.vector.tensor_copy(out=idx_f[:], in_=idx_i[:])

                # idx broadcast row: [128, 512], free position = entity id
                row_ps = ppool1.tile([128, N], F32, tag="row_ps")
                for c in range(NCH):
                    nc.tensor.transpose(
                        out=row_ps[:, c * 128:(c + 1) * 128],
                        in_=idx_f[:, c:c + 1].to_broadcast([128, 128]),
                        identity=id128[:],
                    )
                row_sb = wpool.tile([128, N], F32, tag="row_sb")
                nc.scalar.copy(out=row_sb[:], in_=row_ps[:])

                # selection tiles: sel_c[p, f] = (idx[c*128+p] == idx[f])
                sels = []
                for c in range(NCH):
                    s = wpool.tile([128, N], F32, tag=f"sel{c}")
                    nc.vector.tensor_scalar(
                        out=s[:], in0=row_sb[:], scalar1=idx_f[:, c:c + 1],
                        scalar2=None, op0=mybir.AluOpType.is_equal,
                    )
                    sels.append(s)

                # dupcount over j < f  (strict lower in entity order)
                dup_ps = ppool1.tile([1, N], F32, tag="misc_ps")
                for c in range(NCH):
                    u = wpool.tile([128, N], F32, tag=f"upp{c}")
                    nc.gpsimd.affine_select(
                        out=u[:], in_=sels[c][:], pattern=[[1, N]],
                        base=-(c * 128), channel_multiplier=-1,
                        compare_op=mybir.AluOpType.is_gt, fill=0.0,
                    )
                    nc.tensor.matmul(
                        out=dup_ps[:], lhsT=ones_col[:], rhs=u[:],
                        start=(c == 0), stop=(c == NCH - 1),
                    )

                # first-occurrence keeps its idx, duplicates -> TRASH row
                flagged = wpool.tile([1, N], I32, tag="flagged")
                nc.vector.tensor_scalar(
                    out=flagged[:], in0=dup_ps[:], scalar1=0.0, scalar2=None,
                    op0=mybir.AluOpType.is_gt,
                )
                fixed_row = wpool.tile([1, N], F32, tag="fixed_row")
                nc.vector.tensor_copy(out=fixed_row[:], in_=row_sb[0:1, :])
                nc.vector.copy_predicated(
                    out=fixed_row[:], mask=flagged[:], data=trash_row[:]
                )

                # wrap to ucode idx layout: [16, 32] (idx k at [k%16, k//16]),
                # replicated across 16-partition blocks. m2 holds the 512-row
                # twice side by side; its transpose stacks two wrapped copies.
                m2 = wpool.tile([32, 32], F32, tag="m2")
                nc.gpsimd.dma_start(out=m2[:, 0:16], in_=fixed_row[:, :])
                nc.gpsimd.dma_start(out=m2[:, 16:32], in_=fixed_row[:, :])
                w_ps = ppool1.tile([32, 32], F32, tag="misc_ps")
                nc.tensor.transpose(out=w_ps[:], in_=m2[:], identity=id32[:])
                idx16 = wpool.tile([128, N // 16], I16, tag="idx16")
                for k in range(4):
                    nc.vector.tensor_copy(
                        out=idx16[32 * k:32 * (k + 1), :], in_=w_ps[:, :]
                    )

                # proj = relu(embT.T @ W + bias) * mask   [128, 4*32]
                proj_ps = ppool.tile([128, 128], F32, tag="proj_ps")
                for c in range(NCH):
                    sl = slice(c * D_OUT, (c + 1) * D_OUT)
                    nc.tensor.matmul(
                        out=proj_ps[:, sl], lhsT=e0[:, c * 128:(c + 1) * 128],
                        rhs=w0[:], start=True, stop=False,
                    )
                    nc.tensor.matmul(
                        out=proj_ps[:, sl], lhsT=e1[:, c * 128:(c + 1) * 128],
                        rhs=w1[:], start=False, stop=False,
                    )
                    nc.tensor.matmul(
                        out=proj_ps[:, sl], lhsT=ones_row[:], rhs=bias_t[:],
                        start=False, stop=True,
                    )
                proj_sb = wpool.tile([128, 128], F32, tag="proj_sb")
                for c in range(NCH):
                    sl = slice(c * D_OUT, (c + 1) * D_OUT)
                    nc.scalar.activation(
                        out=proj_sb[:, sl], in_=proj_ps[:, sl],
                        func=mybir.ActivationFunctionType.Relu,
                        scale=mask_tiles[c][:, b:b + 1],
                    )

                # group totals: comb[i] = sum_j sel[j, i] * proj[j]
                comb_ps = ppool.tile([128, 128], F32, tag="comb_ps")
                for ic in range(NCH):
                    osl = slice(ic * D_OUT, (ic + 1) * D_OUT)
                    for jc in range(NCH):
                        nc.tensor.matmul(
                            out=comb_ps[:, osl],
                            lhsT=sels[jc][:, ic * 128:(ic + 1) * 128],
                            rhs=proj_sb[:, jc * D_OUT:(jc + 1) * D_OUT],
                            start=(jc == 0), stop=(jc == NCH - 1),
                        )
                comb_sb = wpool.tile([128, 128], F32, tag="comb_sb")
                for ic in range(NCH):
                    osl = slice(ic * D_OUT, (ic + 1) * D_OUT)
                    nc.scalar.copy(out=comb_sb[:, osl], in_=comb_ps[:, osl])

                nc.gpsimd.dma_scatter_add(
                    outs[b][:, :D_OUT],
                    comb_sb[:].rearrange("p (c e) -> p c e", e=D_OUT),
                    idx16[:],
                    N,
                    N,
                    D_OUT,
                    elem_step=ESTEP,
                )
    nc.compile()
    return nc


def _get_nc():
    global _NC_CACHE
    if _NC_CACHE is None:
        _NC_CACHE = build_nc()
    return _NC_CACHE


def _prep_in_maps(entity_embeddings, weight, bias, entity_x, entity_y, entity_num):
    emb = np.ascontiguousarray(np.asarray(entity_embeddings, dtype=np.float32))
    x = np.asarray(entity_x, dtype=np.int32)
    y = np.asarray(entity_y, dtype=np.int32)
    en = np.asarray(entity_num, dtype=np.int32)
    wgt = np.ascontiguousarray(np.asarray(weight, dtype=np.float32))
    bia = np.asarray(bias, dtype=np.float32).reshape(1, D_OUT)

    embT = np.ascontiguousarray(emb.transpose(0, 2, 1))          # [B, 256, 512]
    xc = np.ascontiguousarray(x.reshape(B, NCH, 128).transpose(0, 2, 1))
    yc = np.ascontiguousarray(y.reshape(B, NCH, 128).transpose(0, 2, 1))

    in_maps = []
    for core in range(NCORES):
        sl = slice(core * BPC, (core + 1) * BPC)
        in_maps.append({
            "embT": embT[sl],
            "xc": xc[sl],
            "yc": yc[sl],
            "wgt": wgt,
            "bias": bia,
            "entn": en[sl].reshape(BPC, 1),
        })
    return in_maps


def run(inputs, trace=False, trace_cores=None):
    """Run the bass kernel; returns (full_output [B,32,H,W], BassKernelResults)."""
    nc = _get_nc()
    in_maps = _prep_in_maps(
        inputs["entity_embeddings"], inputs["weight"], inputs["bias"],
        inputs["entity_x"], inputs["entity_y"], inputs["entity_num"],
    )
    res = run_bass_kernel_spmd(
        nc, in_maps, core_ids=list(range(NCORES)), trace=trace,
        trace_cores=trace_cores,
    )
    # gather: per-core out{b} [VROWS, 64] -> [B, HW, 32] -> [B, 32, H, W]
    dev = np.empty((B, HW, D_OUT), dtype=np.float32)
    for core in range(NCORES):
        for b in range(BPC):
            dev[core * BPC + b] = res.results[core][f"out{b}"][:HW, :D_OUT]
    full = np.ascontiguousarray(
        dev.reshape(B, HH, WW, D_OUT).transpose(0, 3, 1, 2)
    )
    return full, res


def kernel(**inputs) -> np.ndarray:
    full, _ = run(inputs, trace=False)
    return full


# revision 10
# speedup vs baseline: 1.4921x; 1.4921x over previous
"""Trainium2 Bass kernel for nn_Encoder_88235808129468 (scatter_memory).

reference semantics:
    proj = relu(emb @ W + b)                      # [B, N, 32]
    proj *= (n < entity_num[b])                   # mask padded entities
    out[b, :, y, x] += proj[b, n, :]              # scatter-add into [B, 32, H, W]

Strategy (pure data-parallel over batch: 8 cores x 8 batches):
  - ExternalOutput DRAM buffers are pre-zeroed by the PJRT runner (documented
    contract in bass2jax.run_bass_via_pjrt), so the kernel only writes the
    scattered entity rows; untouched cells stay zero.
  - Device output layout is HW-major [25600, 32] rows per batch; the host
    reorders axes to [B, 32, H, W] at the end.
  - Scatter-add duplicates are resolved with a 512x512 selection matrix
    (idx_i == idx_j, with the padding mask folded in): a matmul against proj
    gives EVERY entity the full group total for its cell, then an overwrite
    indirect-DMA scatter writes the rows. Colliding writes carry identical
    bytes, so write order does not matter.
  - proj and the group totals are computed in transposed [32, N] space so
    the matmuls run at F=512 with few instructions, then PE-transposed back
    to entity-major tiles for the scatter.
"""
import sys
import types

sys.path.insert(0, "/opt/trn_rl_repo")

import numpy as np


def _install_axon_hooks_stub():
    """bass_utils imports antenv.axon_hooks when tracing; give it a no-op."""
    if "antenv.axon_hooks" in sys.modules:
        return
    mod = types.ModuleType("antenv.axon_hooks")
    _state = {"hook": None}
    mod.set_axon_ntff_profile_hook = lambda h: _state.__setitem__("hook", h)
    mod.get_axon_ntff_profile_hook = lambda: _state["hook"]
    sys.modules["antenv.axon_hooks"] = mod


_install_axon_hooks_stub()

from concourse import bass, mybir, bacc  # noqa: E402
from concourse.bass_utils import run_bass_kernel_spmd  # noqa: E402
import concourse.tile as tile  # noqa: E402

# Problem constants (hardcoded per harness contract)
B, N, D_IN, D_OUT = 64, 512, 256, 32
HH, WW = 160, 160
HW = HH * WW           # 25600
NCORES = 8
BPC = B // NCORES      # 8 batches per core
NCH = N // 128         # 4 entity chunks of 128
F32 = mybir.dt.float32
BF16 = mybir.dt.bfloat16
I32 = mybir.dt.int32

# comb matmul dtype: "fp32" (exact) or "bf16split" (hi/lo split, ~1e-5 rel)
COMB_MODE = "fp32"

_NC_CACHE = None


def build_nc():
    nc = bacc.Bacc("TRN2", target_bir_lowering=False, debug=False, num_devices=NCORES)

    embT = nc.dram_tensor("embT", [BPC, D_IN, N], F32, kind="ExternalInput")
    xc = nc.dram_tensor("xc", [BPC, 128, NCH], I32, kind="ExternalInput")
    yc = nc.dram_tensor("yc", [BPC, 128, NCH], I32, kind="ExternalInput")
    wgt = nc.dram_tensor("wgt", [D_IN, D_OUT], F32, kind="ExternalInput")
    bias = nc.dram_tensor("bias", [D_OUT, 1], F32, kind="ExternalInput")
    entn = nc.dram_tensor("entn", [1, BPC], I32, kind="ExternalInput")
    outs = [
        nc.dram_tensor(f"out{b}", [HW, D_OUT], F32, kind="ExternalOutput")
        for b in range(BPC)
    ]

    sel_dt = F32 if COMB_MODE == "fp32" else BF16

    with tile.TileContext(nc) as tc:
        with (
            tc.tile_pool(name="const", bufs=1) as cpool,
            tc.tile_pool(name="io", bufs=2) as iopool,
            tc.tile_pool(name="work", bufs=2) as wpool,
            tc.tile_pool(name="ppool", bufs=2, space="PSUM") as ppool,
        ):
            # ---- per-core constants ----
            from concourse.masks import make_identity
            id32 = cpool.tile([32, 32], F32, tag="id32")
            make_identity(nc, id32[:])
            id128 = cpool.tile([128, 128], F32, tag="id128")
            make_identity(nc, id128[:])

            w0 = cpool.tile([128, D_OUT], F32, tag="w0")
            w1 = cpool.tile([128, D_OUT], F32, tag="w1")
            nc.sync.dma_start(out=w0[:], in_=wgt[0:128, :])
            nc.sync.dma_start(out=w1[:], in_=wgt[128:256, :])
            bias_c = cpool.tile([D_OUT, 1], F32, tag="bias_c")
            nc.sync.dma_start(out=bias_c[:], in_=bias[:, :])

            # entity_num broadcast to all partitions via replicating DMA
            entnb = cpool.tile([128, BPC], I32, tag="entnb")
            nc.gpsimd.dma_start(
                out=entnb[:], in_=entn[:, :].to_broadcast([128, BPC])
            )
            mask_tiles = []
            for c in range(NCH):
                iota_c = cpool.tile([128, BPC], I32, tag=f"iota{c}")
                nc.gpsimd.iota(
                    iota_c[:], pattern=[[0, BPC]], base=c * 128, channel_multiplier=1
                )
                m = cpool.tile([128, BPC], F32, tag=f"mask{c}")
                nc.vector.tensor_tensor(
                    out=m[:], in0=iota_c[:], in1=entnb[:], op=mybir.AluOpType.is_lt
                )
                mask_tiles.append(m)

            # ---- per-batch pipeline ----
            for b in range(BPC):
                e0 = iopool.tile([128, N], F32, tag="embT0")
                e1 = iopool.tile([128, N], F32, tag="embT1")
                nc.sync.dma_start(out=e0[:], in_=embT[b, 0:128, :])
                nc.sync.dma_start(out=e1[:], in_=embT[b, 128:256, :])
                xt = iopool.tile([128, NCH], I32, tag="xt")
                yt = iopool.tile([128, NCH], I32, tag="yt")
                nc.sync.dma_start(out=xt[:], in_=xc[b, :, :])
                nc.sync.dma_start(out=yt[:], in_=yc[b, :, :])

                # flat idx, chunk layout [128, 4] (scatter offsets + sel scalars)
                idx_i = wpool.tile([128, NCH], I32, tag="idx_i")
                nc.vector.tensor_scalar(
                    out=idx_i[:], in0=yt[:], scalar1=WW, scalar2=None,
                    op0=mybir.AluOpType.mult,
                )
                nc.vector.tensor_tensor(
                    out=idx_i[:], in0=idx_i[:], in1=xt[:], op=mybir.AluOpType.add
                )
                idx_f = wpool.tile([128, NCH], F32, tag="idx_f")
                nc.vector.tensor_copy(out=idx_f[:], in_=idx_i[:])
                # idx broadcast row [128, 512] via PE transpose of bcast cols
                row_ps = ppool.tile([128, N], F32, tag="row_ps")
                for c in range(NCH):
                    nc.tensor.transpose(
                        out=row_ps[:, c * 128:(c + 1) * 128],
                        in_=idx_f[:, c:c + 1].to_broadcast([128, 128]),
                        identity=id128[:],
                    )
                row_sb = wpool.tile([128, N], F32, tag="row_sb")
                nc.scalar.copy(out=row_sb[:], in_=row_ps[:])

                # selection tiles with mask folded in:
                # sel_c[p, i] = (idx[c*128+p] == idx[i]) * (c*128+p < entity_num)
                sels = []
                for c in range(NCH):
                    s = wpool.tile([128, N], sel_dt, tag=f"sel{c}")
                    nc.vector.tensor_scalar(
                        out=s[:], in0=row_sb[:], scalar1=idx_f[:, c:c + 1],
                        scalar2=mask_tiles[c][:, b:b + 1],
                        op0=mybir.AluOpType.is_equal, op1=mybir.AluOpType.mult,
                    )
                    sels.append(s)

                # projT = relu(W.T @ embT + bias)  [32, 512]
                projT_ps = ppool.tile([32, N], F32, tag="big32_ps")
                nc.tensor.matmul(out=projT_ps[:], lhsT=w0[:], rhs=e0[:],
                                 start=True, stop=False)
                nc.tensor.matmul(out=projT_ps[:], lhsT=w1[:], rhs=e1[:],
                                 start=False, stop=True)
                projT_sb = wpool.tile([32, N], F32, tag="projT_sb")
                nc.scalar.activation(
                    out=projT_sb[:], in_=projT_ps[:],
                    func=mybir.ActivationFunctionType.Relu, bias=bias_c[:, :1],
                )

                # entity-major proj chunks [128, 4*32] via PE transposes
                proj_ps = ppool.tile([128, 128], F32, tag="entmaj_ps")
                for c in range(NCH):
                    nc.tensor.transpose(
                        out=proj_ps[:, c * D_OUT:(c + 1) * D_OUT],
                        in_=projT_sb[:, c * 128:(c + 1) * 128],
                        identity=id32[:],
                    )
                if COMB_MODE == "fp32":
                    proj_sb = wpool.tile([128, 128], F32, tag="proj_sb")
                    for c in range(NCH):
                        sl = slice(c * D_OUT, (c + 1) * D_OUT)
                        nc.scalar.copy(out=proj_sb[:, sl], in_=proj_ps[:, sl])
                    proj_parts = [(proj_sb, 1.0)]
                else:
                    hi = wpool.tile([128, 128], BF16, tag="proj_hi")
                    lo = wpool.tile([128, 128], BF16, tag="proj_lo")
                    fsb = wpool.tile([128, 128], F32, tag="proj_f")
                    for c in range(NCH):
                        sl = slice(c * D_OUT, (c + 1) * D_OUT)
                        nc.scalar.copy(out=fsb[:, sl], in_=proj_ps[:, sl])
                    nc.vector.tensor_copy(out=hi[:], in_=fsb[:])
                    nc.vector.tensor_tensor(
                        out=lo[:], in0=fsb[:], in1=hi[:], op=mybir.AluOpType.subtract
                    )
                    proj_parts = [(hi, None), (lo, None)]

                # group totals, transposed: combT[o, i] = sum_j proj[j, o] sel[j, i]
                combT_ps = ppool.tile([32, N], F32, tag="big32_ps")
                nmm = len(proj_parts) * NCH
                k = 0
                for part, _ in proj_parts:
                    for jc in range(NCH):
                        nc.tensor.matmul(
                            out=combT_ps[:],
                            lhsT=part[:, jc * D_OUT:(jc + 1) * D_OUT],
                            rhs=sels[jc][:],
                            start=(k == 0), stop=(k == nmm - 1),
                        )
                        k += 1
                combT_sb = wpool.tile([32, N], F32, tag="combT_sb")
                nc.scalar.copy(out=combT_sb[:], in_=combT_ps[:])

                # back to entity-major [128, 4*32] and scatter (overwrite)
                comb_ps = ppool.tile([128, 128], F32, tag="entmaj_ps")
                for c in range(NCH):
                    nc.tensor.transpose(
                        out=comb_ps[:, c * D_OUT:(c + 1) * D_OUT],
                        in_=combT_sb[:, c * 128:(c + 1) * 128],
                        identity=id32[:],
                    )
                comb_sb = wpool.tile([128, 128], F32, tag="comb_sb")
                for c in range(NCH):
                    sl = slice(c * D_OUT, (c + 1) * D_OUT)
                    nc.scalar.copy(out=comb_sb[:, sl], in_=comb_ps[:, sl])

                for c in range(NCH):
                    nc.gpsimd.indirect_dma_start(
                        out=outs[b][:, :],
                        out_offset=bass.IndirectOffsetOnAxis(
                            ap=idx_i[:, c:c + 1], axis=0
                        ),
                        in_=comb_sb[:, c * D_OUT:(c + 1) * D_OUT],
                        in_offset=None,
                    )
    nc.compile()
    return nc


def _get_nc():
    global _NC_CACHE
    if _NC_CACHE is None:
        _NC_CACHE = build_nc()
    return _NC_CACHE


def _prep_in_maps(entity_embeddings, weight, bias, entity_x, entity_y, entity_num):
    emb = np.ascontiguousarray(np.asarray(entity_embeddings, dtype=np.float32))
    x = np.asarray(entity_x, dtype=np.int32)
    y = np.asarray(entity_y, dtype=np.int32)
    en = np.asarray(entity_num, dtype=np.int32)
    wgt = np.ascontiguousarray(np.asarray(weight, dtype=np.float32))
    bia = np.ascontiguousarray(np.asarray(bias, dtype=np.float32).reshape(D_OUT, 1))

    embT = np.ascontiguousarray(emb.transpose(0, 2, 1))          # [B, 256, 512]
    xc = np.ascontiguousarray(x.reshape(B, NCH, 128).transpose(0, 2, 1))
    yc = np.ascontiguousarray(y.reshape(B, NCH, 128).transpose(0, 2, 1))

    in_maps = []
    for core in range(NCORES):
        sl = slice(core * BPC, (core + 1) * BPC)
        in_maps.append({
            "embT": embT[sl],
            "xc": xc[sl],
            "yc": yc[sl],
            "wgt": wgt,
            "bias": bia,
            "entn": en[sl].reshape(1, BPC),
        })
    return in_maps


def run(inputs, trace=False, trace_cores=None):
    """Run the bass kernel; returns (full_output [B,32,H,W], BassKernelResults)."""
    nc = _get_nc()
    in_maps = _prep_in_maps(
        inputs["entity_embeddings"], inputs["weight"], inputs["bias"],
        inputs["entity_x"], inputs["entity_y"], inputs["entity_num"],
    )
    res = run_bass_kernel_spmd(
        nc, in_maps, core_ids=list(range(NCORES)), trace=trace,
        trace_cores=trace_cores,
    )
    dev = np.empty((B, HW, D_OUT), dtype=np.float32)
    for core in range(NCORES):
        for b in range(BPC):
            dev[core * BPC + b] = res.results[core][f"out{b}"]
    full = np.ascontiguousarray(
        dev.reshape(B, HH, WW, D_OUT).transpose(0, 3, 1, 2)
    )
    return full, res


def kernel(**inputs) -> np.ndarray:
    full, _ = run(inputs, trace=False)
    return full


# revision 11
# speedup vs baseline: 1.5445x; 1.0351x over previous
"""Trainium2 Bass kernel for nn_Encoder_88235808129468 (scatter_memory).

reference semantics:
    proj = relu(emb @ W + b)                      # [B, N, 32]
    proj *= (n < entity_num[b])                   # mask padded entities
    out[b, :, y, x] += proj[b, n, :]              # scatter-add into [B, 32, H, W]

Strategy (pure data-parallel over batch: 8 cores x 8 batches):
  - ExternalOutput DRAM buffers are pre-zeroed by the PJRT runner (documented
    contract in bass2jax.run_bass_via_pjrt), so the kernel only writes the
    scattered entity rows; untouched cells stay zero.
  - Device output layout is HW-major [25600, 32] rows per batch; the host
    reorders axes to [B, 32, H, W] at the end.
  - Scatter-add duplicates are resolved with a 512x512 selection matrix
    (idx_i == idx_j, with the padding mask folded in): a matmul against proj
    gives EVERY entity the full group total for its cell, then an overwrite
    indirect-DMA scatter writes the rows. Colliding writes carry identical
    bytes, so write order does not matter.
  - proj and the group totals are computed in transposed [32, N] space so
    the matmuls run at F=512 with few instructions, then PE-transposed back
    to entity-major tiles for the scatter.
"""
import sys
import types

sys.path.insert(0, "/opt/trn_rl_repo")

import numpy as np


def _install_axon_hooks_stub():
    """bass_utils imports antenv.axon_hooks when tracing; give it a no-op."""
    if "antenv.axon_hooks" in sys.modules:
        return
    mod = types.ModuleType("antenv.axon_hooks")
    _state = {"hook": None}
    mod.set_axon_ntff_profile_hook = lambda h: _state.__setitem__("hook", h)
    mod.get_axon_ntff_profile_hook = lambda: _state["hook"]
    sys.modules["antenv.axon_hooks"] = mod


_install_axon_hooks_stub()

from concourse import bass, mybir, bacc  # noqa: E402
from concourse.bass_utils import run_bass_kernel_spmd  # noqa: E402
import concourse.tile as tile  # noqa: E402

# Problem constants (hardcoded per harness contract)
B, N, D_IN, D_OUT = 64, 512, 256, 32
HH, WW = 160, 160
HW = HH * WW           # 25600
NCORES = 8
BPC = B // NCORES      # 8 batches per core
NCH = N // 128         # 4 entity chunks of 128
F32 = mybir.dt.float32
BF16 = mybir.dt.bfloat16
I32 = mybir.dt.int32

# comb matmul dtype: "fp32" (exact) or "bf16split" (hi/lo split, ~1e-5 rel)
COMB_MODE = "bf16split"

_NC_CACHE = None


def build_nc():
    nc = bacc.Bacc("TRN2", target_bir_lowering=False, debug=False, num_devices=NCORES)

    embT = nc.dram_tensor("embT", [BPC, D_IN, N], F32, kind="ExternalInput")
    xc = nc.dram_tensor("xc", [BPC, 128, NCH], I32, kind="ExternalInput")
    yc = nc.dram_tensor("yc", [BPC, 128, NCH], I32, kind="ExternalInput")
    wgt = nc.dram_tensor("wgt", [D_IN, D_OUT], F32, kind="ExternalInput")
    bias = nc.dram_tensor("bias", [D_OUT, 1], F32, kind="ExternalInput")
    entn = nc.dram_tensor("entn", [1, BPC], I32, kind="ExternalInput")
    outs = [
        nc.dram_tensor(f"out{b}", [HW, D_OUT], F32, kind="ExternalOutput")
        for b in range(BPC)
    ]

    sel_dt = F32 if COMB_MODE == "fp32" else BF16

    with tile.TileContext(nc) as tc:
        with (
            tc.tile_pool(name="const", bufs=1) as cpool,
            tc.tile_pool(name="io", bufs=2) as iopool,
            tc.tile_pool(name="work", bufs=2) as wpool,
            tc.tile_pool(name="ppool", bufs=2, space="PSUM") as ppool,
        ):
            # ---- per-core constants ----
            from concourse.masks import make_identity
            id32 = cpool.tile([32, 32], F32, tag="id32")
            make_identity(nc, id32[:])
            id128 = cpool.tile([128, 128], F32, tag="id128")
            make_identity(nc, id128[:])

            w0 = cpool.tile([128, D_OUT], F32, tag="w0")
            w1 = cpool.tile([128, D_OUT], F32, tag="w1")
            nc.sync.dma_start(out=w0[:], in_=wgt[0:128, :])
            nc.sync.dma_start(out=w1[:], in_=wgt[128:256, :])
            bias_c = cpool.tile([D_OUT, 1], F32, tag="bias_c")
            nc.sync.dma_start(out=bias_c[:], in_=bias[:, :])

            # entity_num broadcast to all partitions via replicating DMA
            entnb = cpool.tile([128, BPC], I32, tag="entnb")
            nc.gpsimd.dma_start(
                out=entnb[:], in_=entn[:, :].to_broadcast([128, BPC])
            )
            mask_tiles = []
            for c in range(NCH):
                iota_c = cpool.tile([128, BPC], I32, tag=f"iota{c}")
                nc.gpsimd.iota(
                    iota_c[:], pattern=[[0, BPC]], base=c * 128, channel_multiplier=1
                )
                m = cpool.tile([128, BPC], F32, tag=f"mask{c}")
                nc.vector.tensor_tensor(
                    out=m[:], in0=iota_c[:], in1=entnb[:], op=mybir.AluOpType.is_lt
                )
                mask_tiles.append(m)

            # ---- per-batch pipeline ----
            for b in range(BPC):
                e0 = iopool.tile([128, N], F32, tag="embT0")
                e1 = iopool.tile([128, N], F32, tag="embT1")
                nc.sync.dma_start(out=e0[:], in_=embT[b, 0:128, :])
                nc.sync.dma_start(out=e1[:], in_=embT[b, 128:256, :])
                xt = iopool.tile([128, NCH], I32, tag="xt")
                yt = iopool.tile([128, NCH], I32, tag="yt")
                nc.sync.dma_start(out=xt[:], in_=xc[b, :, :])
                nc.sync.dma_start(out=yt[:], in_=yc[b, :, :])

                # flat idx, chunk layout [128, 4] (scatter offsets + sel scalars)
                idx_i = wpool.tile([128, NCH], I32, tag="idx_i")
                nc.vector.tensor_scalar(
                    out=idx_i[:], in0=yt[:], scalar1=WW, scalar2=None,
                    op0=mybir.AluOpType.mult,
                )
                nc.vector.tensor_tensor(
                    out=idx_i[:], in0=idx_i[:], in1=xt[:], op=mybir.AluOpType.add
                )
                idx_f = wpool.tile([128, NCH], F32, tag="idx_f")
                nc.vector.tensor_copy(out=idx_f[:], in_=idx_i[:])
                # idx broadcast row [128, 512] via PE transpose of bcast cols
                row_ps = ppool.tile([128, N], F32, tag="row_ps")
                for c in range(NCH):
                    nc.tensor.transpose(
                        out=row_ps[:, c * 128:(c + 1) * 128],
                        in_=idx_f[:, c:c + 1].to_broadcast([128, 128]),
                        identity=id128[:],
                    )
                row_sb = wpool.tile([128, N], F32, tag="row_sb")
                nc.vector.tensor_copy(out=row_sb[:], in_=row_ps[:])

                # selection tiles with mask folded in:
                # sel_c[p, i] = (idx[c*128+p] == idx[i]) * (c*128+p < entity_num)
                sels = []
                for c in range(NCH):
                    s = wpool.tile([128, N], sel_dt, tag=f"sel{c}")
                    nc.vector.tensor_scalar(
                        out=s[:], in0=row_sb[:], scalar1=idx_f[:, c:c + 1],
                        scalar2=mask_tiles[c][:, b:b + 1],
                        op0=mybir.AluOpType.is_equal, op1=mybir.AluOpType.mult,
                    )
                    sels.append(s)

                # projT = relu(W.T @ embT + bias)  [32, 512]
                projT_ps = ppool.tile([32, N], F32, tag="big32_ps")
                nc.tensor.matmul(out=projT_ps[:], lhsT=w0[:], rhs=e0[:],
                                 start=True, stop=False)
                nc.tensor.matmul(out=projT_ps[:], lhsT=w1[:], rhs=e1[:],
                                 start=False, stop=True)
                projT_sb = wpool.tile([32, N], F32, tag="projT_sb")
                nc.scalar.activation(
                    out=projT_sb[:], in_=projT_ps[:],
                    func=mybir.ActivationFunctionType.Relu, bias=bias_c[:, :1],
                )

                # entity-major proj chunks [128, 4*32] via PE transposes
                proj_ps = ppool.tile([128, 128], F32, tag="entmaj_ps")
                for c in range(NCH):
                    nc.tensor.transpose(
                        out=proj_ps[:, c * D_OUT:(c + 1) * D_OUT],
                        in_=projT_sb[:, c * 128:(c + 1) * 128],
                        identity=id32[:],
                    )
                if COMB_MODE == "fp32":
                    proj_sb = wpool.tile([128, 128], F32, tag="proj_sb")
                    for c in range(NCH):
                        sl = slice(c * D_OUT, (c + 1) * D_OUT)
                        nc.scalar.copy(out=proj_sb[:, sl], in_=proj_ps[:, sl])
                    proj_parts = [(proj_sb, 1.0)]
                else:
                    hi = wpool.tile([128, 128], BF16, tag="proj_hi")
                    lo = wpool.tile([128, 128], BF16, tag="proj_lo")
                    fsb = wpool.tile([128, 128], F32, tag="proj_f")
                    for c in range(NCH):
                        sl = slice(c * D_OUT, (c + 1) * D_OUT)
                        nc.scalar.copy(out=fsb[:, sl], in_=proj_ps[:, sl])
                    nc.vector.tensor_copy(out=hi[:], in_=fsb[:])
                    nc.vector.tensor_tensor(
                        out=lo[:], in0=fsb[:], in1=hi[:], op=mybir.AluOpType.subtract
                    )
                    proj_parts = [(hi, None), (lo, None)]

                # group totals, transposed: combT[o, i] = sum_j proj[j, o] sel[j, i]
                combT_ps = ppool.tile([32, N], F32, tag="big32_ps")
                nmm = len(proj_parts) * NCH
                k = 0
                for part, _ in proj_parts:
                    for jc in range(NCH):
                        nc.tensor.matmul(
                            out=combT_ps[:],
                            lhsT=part[:, jc * D_OUT:(jc + 1) * D_OUT],
                            rhs=sels[jc][:],
                            start=(k == 0), stop=(k == nmm - 1),
                        )
                        k += 1
                combT_sb = wpool.tile([32, N], F32, tag="combT_sb")
                nc.scalar.copy(out=combT_sb[:], in_=combT_ps[:])

                # back to entity-major [128, 4*32] and scatter (overwrite)
                comb_ps = ppool.tile([128, 128], F32, tag="entmaj_ps")
                for c in range(NCH):
                    nc.tensor.transpose(
                        out=comb_ps[:, c * D_OUT:(c + 1) * D_OUT],
                        in_=combT_sb[:, c * 128:(c + 1) * 128],
                        identity=id32[:],
                    )
                comb_sb = wpool.tile([128, 128], F32, tag="comb_sb")
                for c in range(NCH):
                    sl = slice(c * D_OUT, (c + 1) * D_OUT)
                    nc.scalar.copy(out=comb_sb[:, sl], in_=comb_ps[:, sl])

                for c in range(NCH):
                    nc.gpsimd.indirect_dma_start(
                        out=outs[b][:, :],
                        out_offset=bass.IndirectOffsetOnAxis(
                            ap=idx_i[:, c:c + 1], axis=0
                        ),
                        in_=comb_sb[:, c * D_OUT:(c + 1) * D_OUT],
                        in_offset=None,
                    )
    nc.compile()
    return nc


def _get_nc():
    global _NC_CACHE
    if _NC_CACHE is None:
        _NC_CACHE = build_nc()
    return _NC_CACHE


def _prep_in_maps(entity_embeddings, weight, bias, entity_x, entity_y, entity_num):
    emb = np.ascontiguousarray(np.asarray(entity_embeddings, dtype=np.float32))
    x = np.asarray(entity_x, dtype=np.int32)
    y = np.asarray(entity_y, dtype=np.int32)
    en = np.asarray(entity_num, dtype=np.int32)
    wgt = np.ascontiguousarray(np.asarray(weight, dtype=np.float32))
    bia = np.ascontiguousarray(np.asarray(bias, dtype=np.float32).reshape(D_OUT, 1))

    embT = np.ascontiguousarray(emb.transpose(0, 2, 1))          # [B, 256, 512]
    xc = np.ascontiguousarray(x.reshape(B, NCH, 128).transpose(0, 2, 1))
    yc = np.ascontiguousarray(y.reshape(B, NCH, 128).transpose(0, 2, 1))

    in_maps = []
    for core in range(NCORES):
        sl = slice(core * BPC, (core + 1) * BPC)
        in_maps.append({
            "embT": embT[sl],
            "xc": xc[sl],
            "yc": yc[sl],
            "wgt": wgt,
            "bias": bia,
            "entn": en[sl].reshape(1, BPC),
        })
    return in_maps


def run(inputs, trace=False, trace_cores=None):
    """Run the bass kernel; returns (full_output [B,32,H,W], BassKernelResults)."""
    nc = _get_nc()
    in_maps = _prep_in_maps(
        inputs["entity_embeddings"], inputs["weight"], inputs["bias"],
        inputs["entity_x"], inputs["entity_y"], inputs["entity_num"],
    )
    res = run_bass_kernel_spmd(
        nc, in_maps, core_ids=list(range(NCORES)), trace=trace,
        trace_cores=trace_cores,
    )
    dev = np.empty((B, HW, D_OUT), dtype=np.float32)
    for core in range(NCORES):
        for b in range(BPC):
            dev[core * BPC + b] = res.results[core][f"out{b}"]
    full = np.ascontiguousarray(
        dev.reshape(B, HH, WW, D_OUT).transpose(0, 3, 1, 2)
    )
    return full, res


def kernel(**inputs) -> np.ndarray:
    full, _ = run(inputs, trace=False)
    return full


# revision 12
# speedup vs baseline: 2.0005x; 1.2952x over previous
"""Trainium2 Bass kernel for nn_Encoder_88235808129468 (scatter_memory).

reference semantics:
    proj = relu(emb @ W + b)                      # [B, N, 32]
    proj *= (n < entity_num[b])                   # mask padded entities
    out[b, :, y, x] += proj[b, n, :]              # scatter-add into [B, 32, H, W]

Strategy (pure data-parallel over batch: 8 cores x 8 batches):
  - ExternalOutput DRAM buffers are pre-zeroed by the PJRT runner (documented
    contract in bass2jax.run_bass_via_pjrt), so the kernel only writes the
    scattered entity rows; untouched cells stay zero.
  - Device output layout is HW-major [25600, 32] rows per batch; the host
    reorders axes to [B, 32, H, W] at the end.
  - Scatter-add duplicates are resolved with a 512x512 selection matrix
    (idx_i == idx_j, with the padding mask folded in): a matmul against proj
    gives EVERY entity the full group total for its cell, then an overwrite
    indirect-DMA scatter writes the rows. Colliding writes carry identical
    bytes, so write order does not matter.
  - proj and the group totals are computed in transposed [32, N] space so
    the matmuls run at F=512 with few instructions, then PE-transposed back
    to entity-major tiles for the scatter.
"""
import sys
import types

sys.path.insert(0, "/opt/trn_rl_repo")

import numpy as np


def _install_axon_hooks_stub():
    """bass_utils imports antenv.axon_hooks when tracing; give it a no-op."""
    if "antenv.axon_hooks" in sys.modules:
        return
    mod = types.ModuleType("antenv.axon_hooks")
    _state = {"hook": None}
    mod.set_axon_ntff_profile_hook = lambda h: _state.__setitem__("hook", h)
    mod.get_axon_ntff_profile_hook = lambda: _state["hook"]
    sys.modules["antenv.axon_hooks"] = mod


_install_axon_hooks_stub()

from concourse import bass, mybir, bacc  # noqa: E402
from concourse.bass_utils import run_bass_kernel_spmd  # noqa: E402
import concourse.tile as tile  # noqa: E402

# Problem constants (hardcoded per harness contract)
B, N, D_IN, D_OUT = 64, 512, 256, 32
HH, WW = 160, 160
HW = HH * WW           # 25600
NCORES = 8
BPC = B // NCORES      # 8 batches per core
NCH = N // 128         # 4 entity chunks of 128
F32 = mybir.dt.float32
BF16 = mybir.dt.bfloat16
I32 = mybir.dt.int32

# comb matmul dtype: "fp32" (exact) or "bf16split" (hi/lo split, ~1e-5 rel)
COMB_MODE = "bf16split"

_NC_CACHE = None


def build_nc():
    nc = bacc.Bacc("TRN2", target_bir_lowering=False, debug=False, num_devices=NCORES)

    embT = nc.dram_tensor("embT", [BPC, D_IN, N], F32, kind="ExternalInput")
    xc = nc.dram_tensor("xc", [BPC, 128, NCH], I32, kind="ExternalInput")
    yc = nc.dram_tensor("yc", [BPC, 128, NCH], I32, kind="ExternalInput")
    wgt = nc.dram_tensor("wgt", [D_IN, D_OUT], F32, kind="ExternalInput")
    bias = nc.dram_tensor("bias", [D_OUT, 1], F32, kind="ExternalInput")
    entn = nc.dram_tensor("entn", [1, BPC], I32, kind="ExternalInput")
    outs = [
        nc.dram_tensor(f"out{b}", [HW, D_OUT], F32, kind="ExternalOutput")
        for b in range(BPC)
    ]

    sel_dt = F32 if COMB_MODE == "fp32" else BF16

    with tile.TileContext(nc) as tc:
        with (
            tc.tile_pool(name="const", bufs=1) as cpool,
            tc.tile_pool(name="io", bufs=3) as iopool,
            tc.tile_pool(name="work", bufs=3) as wpool,
            tc.tile_pool(name="ppool", bufs=2, space="PSUM") as ppool,
        ):
            # ---- per-core constants ----
            from concourse.masks import make_identity
            id32 = cpool.tile([32, 32], F32, tag="id32")
            make_identity(nc, id32[:])
            id128 = cpool.tile([128, 128], F32, tag="id128")
            make_identity(nc, id128[:])

            w0 = cpool.tile([128, D_OUT], F32, tag="w0")
            w1 = cpool.tile([128, D_OUT], F32, tag="w1")
            nc.sync.dma_start(out=w0[:], in_=wgt[0:128, :])
            nc.sync.dma_start(out=w1[:], in_=wgt[128:256, :])
            bias_c = cpool.tile([D_OUT, 1], F32, tag="bias_c")
            nc.sync.dma_start(out=bias_c[:], in_=bias[:, :])

            # entity_num broadcast to all partitions via replicating DMA
            entnb = cpool.tile([128, BPC], I32, tag="entnb")
            nc.gpsimd.dma_start(
                out=entnb[:], in_=entn[:, :].to_broadcast([128, BPC])
            )
            mask_tiles = []
            for c in range(NCH):
                iota_c = cpool.tile([128, BPC], I32, tag=f"iota{c}")
                nc.gpsimd.iota(
                    iota_c[:], pattern=[[0, BPC]], base=c * 128, channel_multiplier=1
                )
                m = cpool.tile([128, BPC], F32, tag=f"mask{c}")
                nc.vector.tensor_tensor(
                    out=m[:], in0=iota_c[:], in1=entnb[:], op=mybir.AluOpType.is_lt
                )
                mask_tiles.append(m)

            # ---- per-batch pipeline ----
            for b in range(BPC):
                e0 = iopool.tile([128, N], F32, tag="embT0")
                e1 = iopool.tile([128, N], F32, tag="embT1")
                nc.sync.dma_start(out=e0[:], in_=embT[b, 0:128, :])
                nc.sync.dma_start(out=e1[:], in_=embT[b, 128:256, :])
                xt = iopool.tile([128, NCH], I32, tag="xt")
                yt = iopool.tile([128, NCH], I32, tag="yt")
                nc.sync.dma_start(out=xt[:], in_=xc[b, :, :])
                nc.sync.dma_start(out=yt[:], in_=yc[b, :, :])

                # flat idx, chunk layout [128, 4] (scatter offsets + sel scalars)
                idx_i = wpool.tile([128, NCH], I32, tag="idx_i")
                nc.vector.tensor_scalar(
                    out=idx_i[:], in0=yt[:], scalar1=WW, scalar2=None,
                    op0=mybir.AluOpType.mult,
                )
                nc.vector.tensor_tensor(
                    out=idx_i[:], in0=idx_i[:], in1=xt[:], op=mybir.AluOpType.add
                )
                idx_f = wpool.tile([128, NCH], F32, tag="idx_f")
                nc.vector.tensor_copy(out=idx_f[:], in_=idx_i[:])
                # idx broadcast row [128, 512] via PE transpose of bcast cols
                row_ps = ppool.tile([128, N], F32, tag="row_ps")
                for c in range(NCH):
                    nc.tensor.transpose(
                        out=row_ps[:, c * 128:(c + 1) * 128],
                        in_=idx_f[:, c:c + 1].to_broadcast([128, 128]),
                        identity=id128[:],
                    )
                row_sb = wpool.tile([128, N], F32, tag="row_sb")
                nc.vector.tensor_copy(out=row_sb[:], in_=row_ps[:])

                # selection tiles with mask folded in:
                # sel_c[p, i] = (idx[c*128+p] == idx[i]) * (c*128+p < entity_num)
                sels = []
                for c in range(NCH):
                    s = wpool.tile([128, N], sel_dt, tag=f"sel{c}")
                    nc.vector.tensor_scalar(
                        out=s[:], in0=row_sb[:], scalar1=idx_f[:, c:c + 1],
                        scalar2=mask_tiles[c][:, b:b + 1],
                        op0=mybir.AluOpType.is_equal, op1=mybir.AluOpType.mult,
                    )
                    sels.append(s)

                # projT = relu(W.T @ embT + bias)  [32, 512]
                projT_ps = ppool.tile([32, N], F32, tag="big32_ps")
                nc.tensor.matmul(out=projT_ps[:], lhsT=w0[:], rhs=e0[:],
                                 start=True, stop=False)
                nc.tensor.matmul(out=projT_ps[:], lhsT=w1[:], rhs=e1[:],
                                 start=False, stop=True)
                projT_sb = wpool.tile([32, N], F32, tag="projT_sb")
                nc.scalar.activation(
                    out=projT_sb[:], in_=projT_ps[:],
                    func=mybir.ActivationFunctionType.Relu, bias=bias_c[:, :1],
                )

                # entity-major proj chunks [128, 4*32] via PE transposes
                proj_ps = ppool.tile([128, 128], F32, tag="entmaj_ps")
                for c in range(NCH):
                    nc.tensor.transpose(
                        out=proj_ps[:, c * D_OUT:(c + 1) * D_OUT],
                        in_=projT_sb[:, c * 128:(c + 1) * 128],
                        identity=id32[:],
                    )
                if COMB_MODE == "fp32":
                    proj_sb = wpool.tile([128, 128], F32, tag="proj_sb")
                    for c in range(NCH):
                        sl = slice(c * D_OUT, (c + 1) * D_OUT)
                        nc.scalar.copy(out=proj_sb[:, sl], in_=proj_ps[:, sl])
                    proj_parts = [(proj_sb, 1.0)]
                else:
                    hi = wpool.tile([128, 128], BF16, tag="proj_hi")
                    lo = wpool.tile([128, 128], BF16, tag="proj_lo")
                    fsb = wpool.tile([128, 128], F32, tag="proj_f")
                    nc.scalar.copy(out=fsb[:], in_=proj_ps[:])
                    nc.vector.tensor_copy(out=hi[:], in_=fsb[:])
                    nc.vector.tensor_tensor(
                        out=lo[:], in0=fsb[:], in1=hi[:], op=mybir.AluOpType.subtract
                    )
                    proj_parts = [(hi, None), (lo, None)]

                # group totals, transposed: combT[o, i] = sum_j proj[j, o] sel[j, i]
                combT_ps = ppool.tile([32, N], F32, tag="big32_ps")
                nmm = len(proj_parts) * NCH
                k = 0
                for part, _ in proj_parts:
                    for jc in range(NCH):
                        nc.tensor.matmul(
                            out=combT_ps[:],
                            lhsT=part[:, jc * D_OUT:(jc + 1) * D_OUT],
                            rhs=sels[jc][:],
                            start=(k == 0), stop=(k == nmm - 1),
                        )
                        k += 1
                combT_sb = wpool.tile([32, N], F32, tag="combT_sb")
                nc.vector.tensor_copy(out=combT_sb[:], in_=combT_ps[:])

                # back to entity-major [128, 4*32] and scatter (overwrite)
                comb_ps = ppool.tile([128, 128], F32, tag="entmaj_ps")
                for c in range(NCH):
                    nc.tensor.transpose(
                        out=comb_ps[:, c * D_OUT:(c + 1) * D_OUT],
                        in_=combT_sb[:, c * 128:(c + 1) * 128],
                        identity=id32[:],
                    )
                comb_sb = wpool.tile([128, 128], F32, tag="comb_sb")
                nc.scalar.copy(out=comb_sb[:], in_=comb_ps[:])

                for c in range(NCH):
                    nc.gpsimd.indirect_dma_start(
                        out=outs[b][:, :],
                        out_offset=bass.IndirectOffsetOnAxis(
                            ap=idx_i[:, c:c + 1], axis=0
                        ),
                        in_=comb_sb[:, c * D_OUT:(c + 1) * D_OUT],
                        in_offset=None,
                    )
    nc.compile()
    return nc


def _get_nc():
    global _NC_CACHE
    if _NC_CACHE is None:
        _NC_CACHE = build_nc()
    return _NC_CACHE


def _prep_in_maps(entity_embeddings, weight, bias, entity_x, entity_y, entity_num):
    emb = np.ascontiguousarray(np.asarray(entity_embeddings, dtype=np.float32))
    x = np.asarray(entity_x, dtype=np.int32)
    y = np.asarray(entity_y, dtype=np.int32)
    en = np.asarray(entity_num, dtype=np.int32)
    wgt = np.ascontiguousarray(np.asarray(weight, dtype=np.float32))
    bia = np.ascontiguousarray(np.asarray(bias, dtype=np.float32).reshape(D_OUT, 1))

    embT = np.ascontiguousarray(emb.transpose(0, 2, 1))          # [B, 256, 512]
    xc = np.ascontiguousarray(x.reshape(B, NCH, 128).transpose(0, 2, 1))
    yc = np.ascontiguousarray(y.reshape(B, NCH, 128).transpose(0, 2, 1))

    in_maps = []
    for core in range(NCORES):
        sl = slice(core * BPC, (core + 1) * BPC)
        in_maps.append({
            "embT": embT[sl],
            "xc": xc[sl],
            "yc": yc[sl],
            "wgt": wgt,
            "bias": bia,
            "entn": en[sl].reshape(1, BPC),
        })
    return in_maps


def run(inputs, trace=False, trace_cores=None):
    """Run the bass kernel; returns (full_output [B,32,H,W], BassKernelResults)."""
    nc = _get_nc()
    in_maps = _prep_in_maps(
        inputs["entity_embeddings"], inputs["weight"], inputs["bias"],
        inputs["entity_x"], inputs["entity_y"], inputs["entity_num"],
    )
    res = run_bass_kernel_spmd(
        nc, in_maps, core_ids=list(range(NCORES)), trace=trace,
        trace_cores=trace_cores,
    )
    dev = np.empty((B, HW, D_OUT), dtype=np.float32)
    for core in range(NCORES):
        for b in range(BPC):
            dev[core * BPC + b] = res.results[core][f"out{b}"]
    full = np.ascontiguousarray(
        dev.reshape(B, HH, WW, D_OUT).transpose(0, 3, 1, 2)
    )
    return full, res


def kernel(**inputs) -> np.ndarray:
    full, _ = run(inputs, trace=False)
    return full
